# revision 23
# baseline (speedup 1.0000x reference)
"""GegenbauerKAN layer (alpha=1 -> Chebyshev-U basis) on 8 TRN2 NeuronCores.

Math: y[b,o] = sum_{i,d} C_d(tanh(x[b,i])) * W[i,o,d],  d=0..7,
where C_d are Gegenbauer(alpha=1) = Chebyshev-U polynomials.

Strategy:
  - Data-parallel over batch: each of the 8 cores handles 2048 rows.
  - Transposed layout: the host feeds x^T slices so the contraction
    index i lives on SBUF partitions with no on-device transposes.
  - On-device basis: exact U_d values via Chebyshev addition formulas,
    computed in fp32 with bf16 copies at the matmul boundary:
        t  = tanh(x)            s4 = (2t)^2 = U2+1
        b3 = (s4-2)t = U3/2     q2 = (s4-1)^2 = U2^2
        b4 = q2-s4   = U4       b5 = (s4-2)b3 = (U5+2t)/2
        q3 = b3^2               b6 = 4q3-q2 = U6
        e4 = (s4-2)^2           b7 = (e4-2)b3 = U7/2
  - Matmuls in bf16 (1 cycle/row on the PE, same as f32r, but half the
    weight DMA bytes, which is what the pipeline head is bound by, and
    cheaper LDWEIGHTS so weight loads hide fully under the matmuls).
    x ships as fp16 (adds only ~5e-4 rel err; halves the x stream).
    End-to-end rel err ~2.8e-3 vs the 2e-2 gate.
  - k=0 (U_0 = 1) is folded into a per-output bias computed exactly on
    the HOST in fp64 and DMA'd in as a tiny [128, OT] tensor, added at
    PSUM eviction.
  - k-outer matmul order: all four output-tile PSUM groups accumulate
    simultaneously, so weight tile k is first needed ~3.8us * (k-1) in
    and the weight stream can never stall the PE.
  - Head DMA schedule: per-queue transfers FIFO at ~45-100 GB/s while
    the queues share HBM with the other 7 cores' identical fetch
    pattern, so the critical first transfers are spread over the three
    DMA-capable queues: gpsimd[x0a0, wv1(a01), wv2..wv7],
    sync[wv1(a23), x0a1, x0a3, x1..x3, y stores], scalar[bias, x0a2].
    Chunk 0 consumes a-tiles in arrival order [0,2,1,3] and its basis
    DAG is emitted per-a-subtile with the bf16 tensors computed
    directly from their inputs (1-2 op latency, identical values).
  - Later x chunks live in a bufs=1 pool: chunk c+1's x DMA descriptor
    only issues after chunk c's tanh reads, keeping the x stream off
    the HBM while the weights race in at the head.
  - Junk matmuls at t=0 bridge the HAM p-state window (PE at 1.2 GHz
    until ~3.4us of sustained activity) so real matmuls run at 2.4 GHz.
  - Last chunk runs j-outer with immediate per-j eviction so only one
    [128,512] eviction + store sits in the pipeline tail.
  - Weights basis change on host: y = sum_k phi_k . V_k with
    V[:,:,k] = sum_d W[:,:,d] M[d,k], M the (exact, tiny) change of
    basis from {phi_k} to {U_d}; applied in fp64, rounded once.
"""

import ml_dtypes
import numpy as np

import concourse.bacc as bacc
import concourse.mybir as mybir
import concourse.tile as tile
from concourse.alu_op_type import AluOpType as ALU
from concourse.bass_utils import run_bass_kernel_spmd

F32 = mybir.dt.float32
F16 = mybir.dt.float16
BF16 = mybir.dt.bfloat16
AF = mybir.ActivationFunctionType

N_CORES = 8
B = 16384
I = 512
O = 512
DEG = 8  # degrees 0..7
NK = DEG - 1  # degrees 1..7 computed on device; degree 0 is the host bias
B_LOC = B // N_CORES  # 2048 rows per core
CHUNK = 512  # b columns processed per pipeline stage
N_CHUNKS = B_LOC // CHUNK
IT = I // 128  # 4 partition tiles of the input-feature dim
OT = O // 128  # 4 partition tiles of the output dim
N_WARM = 17  # p-state warm-up matmuls (256 cols each) bridging the
# engine-program start to first-operand arrival (~6-10us, run-varying);
# overshoot costs little since chunk 0 is weight-stream-paced anyway
A0_ORDER = [0, 2, 1, 3]  # chunk-0 a-tile consumption = DMA arrival order


def _basis_matrix() -> np.ndarray:
    """M[d,k]: U_d = sum_k M[d,k] * phi_k for the on-device basis
    phi = [1, t, s4, b3, b4, b5, b6, b7]."""
    M = np.zeros((DEG, DEG))
    M[0, 0] = 1.0
    M[1, 1] = 2.0  # U1 = 2 t
    M[2, 0] = -1.0
    M[2, 2] = 1.0  # U2 = s4 - 1
    M[3, 3] = 2.0  # U3 = 2 b3
    M[4, 4] = 1.0  # U4 = b4
    M[5, 5] = 2.0
    M[5, 1] = -2.0  # U5 = 2 b5 - 2 t
    M[6, 6] = 1.0  # U6 = b6
    M[7, 7] = 2.0  # U7 = 2 b7
    return M


def _build_nc():
    nc = bacc.Bacc("TRN2", target_bir_lowering=False, debug=False)

    xt = nc.dram_tensor("xt", [I, B_LOC], F16, kind="ExternalInput")
    wv = nc.dram_tensor("wv", [NK * I, O], BF16, kind="ExternalInput")
    bias_in = nc.dram_tensor("bias_in", [128, OT], F32, kind="ExternalInput")
    yt = nc.dram_tensor("yt", [O, B_LOC], BF16, kind="ExternalOutput")

    with tile.TileContext(nc) as tc:
        with (
            tc.tile_pool(name="wvp", bufs=1) as wvp,
            tc.tile_pool(name="sb", bufs=1) as sb,
            # bufs=1 is deliberate: chunk c+1's x DMA descriptor can only
            # issue after chunk c's tanh reads, which keeps the x stream
            # off the HBM while the weight tiles race in at the head.
            tc.tile_pool(name="xtp", bufs=1) as xtp,
            tc.tile_pool(name="outp", bufs=4) as outp,
            tc.tile_pool(name="ps", bufs=8, space="PSUM") as ps,
        ):
            # --- t=0 setup: constants + warm-up source on the vector queue
            # (memset cannot target bf16/f32r reliably; memset f32, cast)
            wz_f = sb.tile([128, 256], F32, tag="wz_f")
            nc.vector.memset(wz_f[:], 0.0)
            wz = sb.tile([128, 256], BF16, tag="wz")
            nc.vector.tensor_copy(wz[:], wz_f[:])
            neg1 = sb.tile([128, 1], F32, tag="neg1")
            nc.vector.memset(neg1[:], -1.0)
            neg2 = sb.tile([128, 1], F32, tag="neg2")
            nc.vector.memset(neg2[:], -2.0)

            # Weight tiles (bf16). wv1 lands as two halves on two queues;
            # wv2..wv7 stream whole-tile on gpsimd, arriving just ahead
            # of the k-outer consumer (0.5 MiB / ~3.5us each).
            wv_sb = [None] * (NK + 1)
            for k in range(1, NK + 1):
                wv_sb[k] = wvp.tile(
                    [128, IT, O], BF16, tag=f"wv{k}", name=f"wv{k}_sb"
                )

            def wv_dma(eng, k, alo, ahi):
                r0 = (k - 1) * I + alo * 128
                r1 = (k - 1) * I + ahi * 128
                eng.dma_start(
                    out=wv_sb[k][:, alo:ahi, :],
                    in_=wv[r0:r1, :].rearrange("(a p) o -> p a o", p=128),
                )

            # Only sync/scalar/gpsimd queues can issue DMAs, with
            # per-queue FIFO streams. The two most latency-critical
            # transfers — x0's a0 slice and wv1's first half — lead the
            # gpsimd queue; the rest of x0 rides sync/scalar so no
            # single queue serializes the whole head.
            bias_sb = sb.tile([128, OT], F32, tag="bias")
            x0_sb = xtp.tile([128, IT, CHUNK], F16, tag="xt")

            hc = CHUNK // 2
            nc.gpsimd.dma_start(out=x0_sb[:, 0, 0:hc], in_=xt[0:128, 0:hc])
            nc.scalar.dma_start(out=x0_sb[:, 0, hc:CHUNK], in_=xt[0:128, hc:CHUNK])
            wv_dma(nc.sync, 1, 2, 4)
            wv_dma(nc.gpsimd, 1, 0, 2)
            nc.scalar.dma_start(out=bias_sb[:], in_=bias_in[:, :])
            nc.scalar.dma_start(out=x0_sb[:, 2, :], in_=xt[256:384, 0:CHUNK])
            nc.sync.dma_start(out=x0_sb[:, 1, :], in_=xt[128:256, 0:CHUNK])
            nc.sync.dma_start(out=x0_sb[:, 3, :], in_=xt[384:512, 0:CHUNK])
            for k in range(2, NK + 1):
                wv_dma(nc.gpsimd, k, 0, IT)

            # PE p-state warm-up: junk matmuls on zeros, result never read.
            warm_acc = ps.tile([128, 256], F32, tag="acc")
            for i in range(N_WARM):
                nc.tensor.matmul(
                    warm_acc[:],
                    lhsT=wz[:, 0:128],
                    rhs=wz[:],
                    start=(i == 0),
                    stop=(i == N_WARM - 1),
                )

            # Remaining x chunks on sync; the bufs=1 pool gates each one.
            x_tiles = [x0_sb]
            for c in range(1, N_CHUNKS):
                x_sb = xtp.tile([128, IT, CHUNK], F16, tag="xt", name=f"x{c}_sb")
                nc.sync.dma_start(
                    out=x_sb[:],
                    in_=xt[:, c * CHUNK : (c + 1) * CHUNK].rearrange(
                        "(a p) b -> p a b", p=128
                    ),
                )
                x_tiles.append(x_sb)

            flat = [128, IT * CHUNK]

            def dag(x_sb, fine):
                """fp32 basis DAG with bf16 boundary tensors; fine=True
                splits the early ops per a-subtile (in DMA arrival order)
                and computes the bf16 tensors directly from their inputs
                (identical values to the cast-after path, but 1-2 op
                latency) so the PE never waits at the head."""
                t = sb.tile(flat, F32, tag="t")
                t_r = sb.tile(flat, BF16, tag="t_r", bufs=2)
                s4 = sb.tile(flat, F32, tag="s4")
                s4_r = sb.tile(flat, BF16, tag="s4_r", bufs=2)
                b3 = sb.tile(flat, F32, tag="b3")
                b3_r = sb.tile(flat, BF16, tag="b3_r")

                def sl(ap, a):
                    return ap[:] if a is None else ap[:, a * CHUNK : (a + 1) * CHUNK]

                xf = x_sb[:].rearrange("p a b -> p (a b)")

                if fine:
                    A = A0_ORDER
                    hc = CHUNK // 2
                    # tanh/s4r pairs per a-tile: k=2's operand lands 2 ops
                    # after each x sub-tile arrives. a0 is further split
                    # into column halves so tanh starts after only 64 KiB.
                    for a in A:
                        if a == 0:
                            nc.scalar.activation(t[:, 0:hc], x_sb[:, 0, 0:hc], AF.Tanh)
                            nc.scalar.activation(
                                t[:, hc : 2 * hc], x_sb[:, 0, hc : 2 * hc], AF.Tanh
                            )
                        else:
                            nc.scalar.activation(sl(t, a), x_sb[:, a, :], AF.Tanh)
                        nc.scalar.activation(
                            sl(s4_r, a), sl(t, a), AF.Square, scale=2.0
                        )
                    for a in A:
                        nc.scalar.activation(sl(s4, a), sl(t, a), AF.Square, scale=2.0)
                    nc.vector.tensor_copy(t_r[:, 0:hc], t[:, 0:hc])
                    for a in A:
                        if a == 0:
                            nc.vector.tensor_copy(t_r[:, hc : 2 * hc], t[:, hc : 2 * hc])
                        else:
                            nc.vector.tensor_copy(sl(t_r, a), sl(t, a))
                    for a in A:
                        nc.vector.scalar_tensor_tensor(
                            sl(b3_r, a), sl(s4, a), 2.0, sl(t, a), ALU.subtract, ALU.mult
                        )
                    nc.vector.scalar_tensor_tensor(
                        b3[:], s4[:], 2.0, t[:], ALU.subtract, ALU.mult
                    )
                else:
                    nc.scalar.activation(t[:], xf, AF.Tanh)
                    nc.vector.tensor_copy(t_r[:], t[:])
                    nc.scalar.activation(s4[:], t[:], AF.Square, scale=2.0)
                    nc.vector.tensor_copy(s4_r[:], s4[:])
                    nc.vector.scalar_tensor_tensor(
                        b3[:], s4[:], 2.0, t[:], ALU.subtract, ALU.mult
                    )
                    nc.vector.tensor_copy(b3_r[:], b3[:])
                q2 = sb.tile(flat, F32, tag="q2")
                nc.scalar.activation(q2[:], s4[:], AF.Square, bias=neg1[:])
                b4_r = sb.tile(flat, BF16, tag="b4_r")
                nc.vector.tensor_sub(b4_r[:], q2[:], s4[:])
                b5 = sb.tile(flat, BF16, tag="b5")
                nc.vector.scalar_tensor_tensor(
                    b5[:], s4[:], 2.0, b3[:], ALU.subtract, ALU.mult
                )
                q3 = sb.tile(flat, F32, tag="qe")
                nc.scalar.activation(q3[:], b3[:], AF.Square)
                b6 = sb.tile(flat, BF16, tag="b6")
                nc.vector.scalar_tensor_tensor(
                    b6[:], q3[:], 4.0, q2[:], ALU.mult, ALU.subtract
                )
                e4 = sb.tile(flat, F32, tag="qe")
                nc.scalar.activation(e4[:], s4[:], AF.Square, bias=neg2[:])
                b7 = sb.tile(flat, BF16, tag="b7")
                nc.vector.scalar_tensor_tensor(
                    b7[:], e4[:], 2.0, b3[:], ALU.subtract, ALU.mult
                )
                return [t_r, s4_r, b3_r, b4_r, b5, b6, b7]  # k = 1..7

            pending = []

            def emit_evictions():
                # Evict the previous chunk's PSUM groups. Emitted AFTER the
                # next chunk's basis DAG so the strict-FIFO ACT queue
                # prioritizes producing the basis the PE is waiting on.
                for c0, j, acc in pending:
                    o_sb = outp.tile([128, CHUNK], BF16, tag="out")
                    nc.scalar.activation(
                        o_sb[:], acc[:], AF.Identity, bias=bias_sb[:, j : j + 1]
                    )
                    nc.sync.dma_start(
                        out=yt[j * 128 : (j + 1) * 128, c0 * CHUNK : (c0 + 1) * CHUNK],
                        in_=o_sb[:],
                    )
                pending.clear()

            n_mm = NK * IT
            for c in range(N_CHUNKS):
                basis = dag(x_tiles[c], fine=(c == 0))
                emit_evictions()
                a_order = A0_ORDER if c == 0 else list(range(IT))
                if c < N_CHUNKS - 1:
                    # k-outer: all four output tiles accumulate at once, so
                    # weight tile k is first needed ~3.8us * (k-1) in.
                    accs = [
                        ps.tile([128, CHUNK], F32, tag="acc", name=f"acc_{c}_{j}")
                        for j in range(OT)
                    ]
                    for k in range(1, NK + 1):
                        pk = basis[k - 1]
                        for a in a_order:
                            for j in range(OT):
                                nc.tensor.matmul(
                                    accs[j][:],
                                    lhsT=wv_sb[k][:, a, j * 128 : (j + 1) * 128],
                                    rhs=pk[:, a * CHUNK : (a + 1) * CHUNK],
                                    start=(k == 1 and a == a_order[0]),
                                    stop=(k == NK and a == a_order[-1]),
                                )
                    pending.extend((c, j, accs[j]) for j in range(OT))
                else:
                    # Last chunk: j-outer with immediate eviction so the
                    # tail after the final matmul is one evict + one store.
                    for j in range(OT):
                        acc = ps.tile([128, CHUNK], F32, tag="acc", name=f"acc_{c}_{j}")
                        idx = 0
                        for k in range(1, NK + 1):
                            pk = basis[k - 1]
                            for a in a_order:
                                nc.tensor.matmul(
                                    acc[:],
                                    lhsT=wv_sb[k][:, a, j * 128 : (j + 1) * 128],
                                    rhs=pk[:, a * CHUNK : (a + 1) * CHUNK],
                                    start=(idx == 0),
                                    stop=(idx == n_mm - 1),
                                )
                                idx += 1
                        o_sb = outp.tile([128, CHUNK], BF16, tag="out")
                        if j < OT - 1:
                            nc.scalar.activation(
                                o_sb[:], acc[:], AF.Identity, bias=bias_sb[:, j : j + 1]
                            )
                            nc.sync.dma_start(
                                out=yt[
                                    j * 128 : (j + 1) * 128,
                                    c * CHUNK : (c + 1) * CHUNK,
                                ],
                                in_=o_sb[:],
                            )
                        else:
                            # Very last group: evict + store as two halves on
                            # two queues so the final transfers overlap.
                            cuts = [0, 192, 384, CHUNK]
                            engs = [nc.gpsimd, nc.sync, nc.scalar]
                            for (lo, hi2), eng in zip(zip(cuts, cuts[1:]), engs):
                                nc.scalar.activation(
                                    o_sb[:, lo:hi2],
                                    acc[:, lo:hi2],
                                    AF.Identity,
                                    bias=bias_sb[:, j : j + 1],
                                )
                                eng.dma_start(
                                    out=yt[
                                        j * 128 : (j + 1) * 128,
                                        c * CHUNK + lo : c * CHUNK + hi2,
                                    ],
                                    in_=o_sb[:, lo:hi2],
                                )

    nc.compile()
    return nc


_NC_CACHE = None
_last_in_maps = None


def _get_nc():
    global _NC_CACHE
    if _NC_CACHE is None:
        _NC_CACHE = _build_nc()
    return _NC_CACHE


def kernel(x: np.ndarray, gegenbauer_coeffs: np.ndarray, **unused) -> np.ndarray:
    x = np.asarray(x, dtype=np.float32).reshape(B, I)
    coeffs = np.asarray(gegenbauer_coeffs, dtype=np.float32)

    # Host prep: basis change (exact integers, applied in fp64) and layouts.
    M = _basis_matrix()
    v = np.einsum("iod,dk->kio", coeffs.astype(np.float64), M)
    wv_np = np.ascontiguousarray(
        v[1:].reshape(NK * I, O).astype(np.float32).astype(ml_dtypes.bfloat16)
    )
    # Degree-0 term: y += sum_i V[i,o,0]; exact in fp64.
    bias_np = np.ascontiguousarray(
        v[0].sum(axis=0).astype(np.float32).reshape(OT, 128).T
    )
    xt_full = np.ascontiguousarray(x.T.astype(np.float16))  # [I, B], fp16: x rounding adds only ~5e-4 rel err

    in_maps = []
    for c in range(N_CORES):
        xt_c = np.ascontiguousarray(xt_full[:, c * B_LOC : (c + 1) * B_LOC])
        in_maps.append({"xt": xt_c, "wv": wv_np, "bias_in": bias_np})

    global _last_in_maps
    _last_in_maps = in_maps

    nc = _get_nc()
    try:
        res = run_bass_kernel_spmd(nc, in_maps, core_ids=list(range(N_CORES)))
    except Exception:
        # A previous crashed session can leave a core unrecoverable until
        # the runtime resets it; one retry clears it.
        res = run_bass_kernel_spmd(nc, in_maps, core_ids=list(range(N_CORES)))

    y = np.empty((B, O), dtype=np.float32)
    for c in range(N_CORES):
        y[c * B_LOC : (c + 1) * B_LOC, :] = res.results[c]["yt"].T.astype(np.float32)
    return y


# revision 25
# speedup vs baseline: 1.1654x; 1.1654x over previous
"""GegenbauerKAN layer (alpha=1 -> Chebyshev-U basis) on 8 TRN2 NeuronCores.

Math: y[b,o] = sum_{i,d} C_d(tanh(x[b,i])) * W[i,o,d],  d=0..7,
where C_d are Gegenbauer(alpha=1) = Chebyshev-U polynomials.

Strategy:
  - Data-parallel over batch: each of the 8 cores handles 2048 rows.
  - Transposed layout: the host feeds x^T slices so the contraction
    index i lives on SBUF partitions with no on-device transposes.
  - On-device basis: exact U_d values via Chebyshev addition formulas,
    computed in fp32 with bf16 copies at the matmul boundary:
        t  = tanh(x)            s4 = (2t)^2 = U2+1
        b3 = (s4-2)t = U3/2     q2 = (s4-1)^2 = U2^2
        b4 = q2-s4   = U4       b5 = (s4-2)b3 = (U5+2t)/2
        q3 = b3^2               b6 = 4q3-q2 = U6
        e4 = (s4-2)^2           b7 = (e4-2)b3 = U7/2
  - Matmuls in bf16 (1 cycle/row on the PE, same as f32r, but half the
    weight DMA bytes, which is what the pipeline head is bound by, and
    cheaper LDWEIGHTS so weight loads hide fully under the matmuls).
    x ships as fp16 (adds only ~5e-4 rel err; halves the x stream).
    End-to-end rel err ~2.8e-3 vs the 2e-2 gate.
  - k=0 (U_0 = 1) is folded into a per-output bias computed exactly on
    the HOST in fp64 and DMA'd in as a tiny [128, OT] tensor, added at
    PSUM eviction.
  - k-outer matmul order: all four output-tile PSUM groups accumulate
    simultaneously, so weight tile k is first needed ~3.8us * (k-1) in
    and the weight stream can never stall the PE.
  - Head DMA schedule: per-queue transfers FIFO at ~45-100 GB/s while
    the queues share HBM with the other 7 cores' identical fetch
    pattern, so the critical first transfers are spread over the three
    DMA-capable queues: gpsimd[x0a0, wv1(a01), wv2..wv7],
    sync[wv1(a23), x0a1, x0a3, x1..x3, y stores], scalar[bias, x0a2].
    Chunk 0 consumes a-tiles in arrival order [0,2,1,3] and its basis
    DAG is emitted per-a-subtile with the bf16 tensors computed
    directly from their inputs (1-2 op latency, identical values).
  - Later x chunks live in a bufs=1 pool: chunk c+1's x DMA descriptor
    only issues after chunk c's tanh reads, keeping the x stream off
    the HBM while the weights race in at the head.
  - Junk matmuls at t=0 bridge the HAM p-state window (PE at 1.2 GHz
    until ~3.4us of sustained activity) so real matmuls run at 2.4 GHz.
  - Last chunk runs j-outer with immediate per-j eviction so only one
    [128,512] eviction + store sits in the pipeline tail.
  - Weights basis change on host: y = sum_k phi_k . V_k with
    V[:,:,k] = sum_d W[:,:,d] M[d,k], M the (exact, tiny) change of
    basis from {phi_k} to {U_d}; applied in fp64, rounded once.
"""

import ml_dtypes
import numpy as np

import concourse.bacc as bacc
import concourse.mybir as mybir
import concourse.tile as tile
from concourse.alu_op_type import AluOpType as ALU
from concourse.bass_utils import run_bass_kernel_spmd

F32 = mybir.dt.float32
F16 = mybir.dt.float16
BF16 = mybir.dt.bfloat16
AF = mybir.ActivationFunctionType

N_CORES = 8
B = 16384
I = 512
O = 512
DEG = 8  # degrees 0..7
NK = DEG - 1  # degrees 1..7 computed on device; degree 0 is the host bias
B_LOC = B // N_CORES  # 2048 rows per core
CHUNK = 512  # b columns processed per pipeline stage
N_CHUNKS = B_LOC // CHUNK
IT = I // 128  # 4 partition tiles of the input-feature dim
OT = O // 128  # 4 partition tiles of the output dim
N_WARM = 17  # p-state warm-up matmuls (256 cols each) bridging the
# engine-program start to first-operand arrival (~6-10us, run-varying);
# overshoot costs little since chunk 0 is weight-stream-paced anyway
A0_ORDER = [0, 2, 1, 3]  # chunk-0 a-tile consumption = DMA arrival order


def _basis_matrix() -> np.ndarray:
    """M[d,k]: U_d = sum_k M[d,k] * phi_k for the on-device basis
    phi = [1, t, s4, b3, b4, b5, b6, b7]."""
    M = np.zeros((DEG, DEG))
    M[0, 0] = 1.0
    M[1, 1] = 2.0  # U1 = 2 t
    M[2, 0] = -1.0
    M[2, 2] = 1.0  # U2 = s4 - 1
    M[3, 3] = 2.0  # U3 = 2 b3
    M[4, 4] = 1.0  # U4 = b4
    M[5, 5] = 2.0
    M[5, 1] = -2.0  # U5 = 2 b5 - 2 t
    M[6, 6] = 1.0  # U6 = b6
    M[7, 7] = 2.0  # U7 = 2 b7
    return M


def _build_nc():
    nc = bacc.Bacc("TRN2", target_bir_lowering=False, debug=False)

    xt = nc.dram_tensor("xt", [I, B_LOC], F16, kind="ExternalInput")
    wv = nc.dram_tensor("wv", [NK * I, O], BF16, kind="ExternalInput")
    bias_in = nc.dram_tensor("bias_in", [128, OT], F32, kind="ExternalInput")
    yt = nc.dram_tensor("yt", [O, B_LOC], BF16, kind="ExternalOutput")

    with tile.TileContext(nc) as tc:
        with (
            tc.tile_pool(name="wvp", bufs=1) as wvp,
            tc.tile_pool(name="sb", bufs=1) as sb,
            # bufs=1 is deliberate: chunk c+1's x DMA descriptor can only
            # issue after chunk c's tanh reads, which keeps the x stream
            # off the HBM while the weight tiles race in at the head.
            tc.tile_pool(name="xtp", bufs=1) as xtp,
            tc.tile_pool(name="outp", bufs=4) as outp,
            tc.tile_pool(name="ps", bufs=8, space="PSUM") as ps,
        ):
            # --- t=0 setup: constants + warm-up source on the vector queue
            # (memset cannot target bf16/f32r reliably; memset f32, cast)
            wz_f = sb.tile([128, 256], F32, tag="wz_f")
            nc.vector.memset(wz_f[:], 0.0)
            wz = sb.tile([128, 256], BF16, tag="wz")
            nc.vector.tensor_copy(wz[:], wz_f[:])
            neg1 = sb.tile([128, 1], F32, tag="neg1")
            nc.vector.memset(neg1[:], -1.0)
            neg2 = sb.tile([128, 1], F32, tag="neg2")
            nc.vector.memset(neg2[:], -2.0)

            # Weight tiles (bf16). wv1 lands as two halves on two queues;
            # wv2..wv7 stream whole-tile on gpsimd, arriving just ahead
            # of the k-outer consumer (0.5 MiB / ~3.5us each).
            wv_sb = [None] * (NK + 1)
            for k in range(1, NK + 1):
                wv_sb[k] = wvp.tile(
                    [128, IT, O], BF16, tag=f"wv{k}", name=f"wv{k}_sb"
                )

            def wv_dma(eng, k, alo, ahi):
                r0 = (k - 1) * I + alo * 128
                r1 = (k - 1) * I + ahi * 128
                eng.dma_start(
                    out=wv_sb[k][:, alo:ahi, :],
                    in_=wv[r0:r1, :].rearrange("(a p) o -> p a o", p=128),
                )

            # Only sync/scalar/gpsimd queues can issue DMAs, with
            # per-queue FIFO streams. The two most latency-critical
            # transfers — x0's a0 slice and wv1's first half — lead the
            # gpsimd queue; the rest of x0 rides sync/scalar so no
            # single queue serializes the whole head.
            bias_sb = sb.tile([128, OT], F32, tag="bias")
            x0_sb = xtp.tile([128, IT, CHUNK], F16, tag="xt")

            hc = CHUNK // 2
            nc.gpsimd.dma_start(out=x0_sb[:, 0, 0:hc], in_=xt[0:128, 0:hc])
            nc.scalar.dma_start(out=x0_sb[:, 0, hc:CHUNK], in_=xt[0:128, hc:CHUNK])
            wv_dma(nc.sync, 1, 2, 4)
            wv_dma(nc.gpsimd, 1, 0, 2)
            nc.scalar.dma_start(out=bias_sb[:], in_=bias_in[:, :])
            nc.scalar.dma_start(out=x0_sb[:, 2, :], in_=xt[256:384, 0:CHUNK])
            nc.sync.dma_start(out=x0_sb[:, 1, :], in_=xt[128:256, 0:CHUNK])
            nc.sync.dma_start(out=x0_sb[:, 3, :], in_=xt[384:512, 0:CHUNK])
            for k in range(2, NK + 1):
                wv_dma(nc.gpsimd, k, 0, IT)

            # PE p-state warm-up: junk matmuls on zeros, result never read.
            warm_acc = ps.tile([128, 256], F32, tag="acc")
            for i in range(N_WARM):
                nc.tensor.matmul(
                    warm_acc[:],
                    lhsT=wz[:, 0:128],
                    rhs=wz[:],
                    start=(i == 0),
                    stop=(i == N_WARM - 1),
                )

            # Remaining x chunks on sync; the bufs=1 pool gates each one.
            x_tiles = [x0_sb]
            for c in range(1, N_CHUNKS):
                x_sb = xtp.tile([128, IT, CHUNK], F16, tag="xt", name=f"x{c}_sb")
                nc.sync.dma_start(
                    out=x_sb[:],
                    in_=xt[:, c * CHUNK : (c + 1) * CHUNK].rearrange(
                        "(a p) b -> p a b", p=128
                    ),
                )
                x_tiles.append(x_sb)

            flat = [128, IT * CHUNK]

            def dag(x_sb, fine):
                """fp32 basis DAG with bf16 boundary tensors; fine=True
                splits the early ops per a-subtile (in DMA arrival order)
                and computes the bf16 tensors directly from their inputs
                (identical values to the cast-after path, but 1-2 op
                latency) so the PE never waits at the head."""
                t = sb.tile(flat, F32, tag="t")
                t_r = sb.tile(flat, BF16, tag="t_r", bufs=2)
                s4 = sb.tile(flat, F32, tag="s4")
                s4_r = sb.tile(flat, BF16, tag="s4_r", bufs=2)
                b3 = sb.tile(flat, F32, tag="b3")
                b3_r = sb.tile(flat, BF16, tag="b3_r")

                def sl(ap, a):
                    return ap[:] if a is None else ap[:, a * CHUNK : (a + 1) * CHUNK]

                xf = x_sb[:].rearrange("p a b -> p (a b)")

                if fine:
                    A = A0_ORDER
                    hc = CHUNK // 2
                    # tanh/s4r pairs per a-tile: k=2's operand lands 2 ops
                    # after each x sub-tile arrives. a0 is further split
                    # into column halves so tanh starts after only 64 KiB.
                    for a in A:
                        if a == 0:
                            nc.scalar.activation(t[:, 0:hc], x_sb[:, 0, 0:hc], AF.Tanh)
                            nc.scalar.activation(
                                t[:, hc : 2 * hc], x_sb[:, 0, hc : 2 * hc], AF.Tanh
                            )
                        else:
                            nc.scalar.activation(sl(t, a), x_sb[:, a, :], AF.Tanh)
                        nc.scalar.activation(
                            sl(s4_r, a), sl(t, a), AF.Square, scale=2.0
                        )
                    for a in A:
                        nc.scalar.activation(sl(s4, a), sl(t, a), AF.Square, scale=2.0)
                    nc.vector.tensor_copy(t_r[:, 0:hc], t[:, 0:hc])
                    for a in A:
                        if a == 0:
                            nc.vector.tensor_copy(t_r[:, hc : 2 * hc], t[:, hc : 2 * hc])
                        else:
                            nc.vector.tensor_copy(sl(t_r, a), sl(t, a))
                    for a in A:
                        nc.vector.scalar_tensor_tensor(
                            sl(b3_r, a), sl(s4, a), 2.0, sl(t, a), ALU.subtract, ALU.mult
                        )
                    nc.vector.scalar_tensor_tensor(
                        b3[:], s4[:], 2.0, t[:], ALU.subtract, ALU.mult
                    )
                else:
                    nc.scalar.activation(t[:], xf, AF.Tanh)
                    nc.vector.tensor_copy(t_r[:], t[:])
                    nc.scalar.activation(s4[:], t[:], AF.Square, scale=2.0)
                    nc.vector.tensor_copy(s4_r[:], s4[:])
                    nc.vector.scalar_tensor_tensor(
                        b3[:], s4[:], 2.0, t[:], ALU.subtract, ALU.mult
                    )
                    nc.vector.tensor_copy(b3_r[:], b3[:])
                q2 = sb.tile(flat, F32, tag="q2")
                nc.scalar.activation(q2[:], s4[:], AF.Square, bias=neg1[:])
                b4_r = sb.tile(flat, BF16, tag="b4_r")
                nc.vector.tensor_sub(b4_r[:], q2[:], s4[:])
                b5 = sb.tile(flat, BF16, tag="b5")
                nc.vector.scalar_tensor_tensor(
                    b5[:], s4[:], 2.0, b3[:], ALU.subtract, ALU.mult
                )
                q3 = sb.tile(flat, F32, tag="qe")
                nc.scalar.activation(q3[:], b3[:], AF.Square)
                b6 = sb.tile(flat, BF16, tag="b6")
                nc.vector.scalar_tensor_tensor(
                    b6[:], q3[:], 4.0, q2[:], ALU.mult, ALU.subtract
                )
                e4 = sb.tile(flat, F32, tag="qe")
                nc.scalar.activation(e4[:], s4[:], AF.Square, bias=neg2[:])
                b7 = sb.tile(flat, BF16, tag="b7")
                nc.vector.scalar_tensor_tensor(
                    b7[:], e4[:], 2.0, b3[:], ALU.subtract, ALU.mult
                )
                return [t_r, s4_r, b3_r, b4_r, b5, b6, b7]  # k = 1..7

            pending = []

            def emit_evictions():
                # Evict the previous chunk's PSUM groups. Emitted AFTER the
                # next chunk's basis DAG so the strict-FIFO ACT queue
                # prioritizes producing the basis the PE is waiting on.
                for c0, j, acc in pending:
                    o_sb = outp.tile([128, CHUNK], BF16, tag="out")
                    nc.scalar.activation(
                        o_sb[:], acc[:], AF.Identity, bias=bias_sb[:, j : j + 1]
                    )
                    nc.sync.dma_start(
                        out=yt[j * 128 : (j + 1) * 128, c0 * CHUNK : (c0 + 1) * CHUNK],
                        in_=o_sb[:],
                    )
                pending.clear()

            n_mm = NK * IT
            for c in range(N_CHUNKS):
                basis = dag(x_tiles[c], fine=(c == 0))
                emit_evictions()
                a_order = A0_ORDER if c == 0 else list(range(IT))
                if c < N_CHUNKS - 1:
                    # k-outer: all four output tiles accumulate at once, so
                    # weight tile k is first needed ~3.8us * (k-1) in.
                    accs = [
                        ps.tile([128, CHUNK], F32, tag="acc", name=f"acc_{c}_{j}")
                        for j in range(OT)
                    ]
                    for k in range(1, NK + 1):
                        pk = basis[k - 1]
                        for a in a_order:
                            for j in range(OT):
                                nc.tensor.matmul(
                                    accs[j][:],
                                    lhsT=wv_sb[k][:, a, j * 128 : (j + 1) * 128],
                                    rhs=pk[:, a * CHUNK : (a + 1) * CHUNK],
                                    start=(k == 1 and a == a_order[0]),
                                    stop=(k == NK and a == a_order[-1]),
                                )
                    pending.extend((c, j, accs[j]) for j in range(OT))
                else:
                    # Last chunk: j-outer with immediate eviction so the
                    # tail after the final matmul is one evict + one store.
                    for j in range(OT):
                        acc = ps.tile([128, CHUNK], F32, tag="acc", name=f"acc_{c}_{j}")
                        idx = 0
                        for k in range(1, NK + 1):
                            pk = basis[k - 1]
                            for a in a_order:
                                nc.tensor.matmul(
                                    acc[:],
                                    lhsT=wv_sb[k][:, a, j * 128 : (j + 1) * 128],
                                    rhs=pk[:, a * CHUNK : (a + 1) * CHUNK],
                                    start=(idx == 0),
                                    stop=(idx == n_mm - 1),
                                )
                                idx += 1
                        o_sb = outp.tile([128, CHUNK], BF16, tag="out")
                        if j < OT - 1:
                            nc.scalar.activation(
                                o_sb[:], acc[:], AF.Identity, bias=bias_sb[:, j : j + 1]
                            )
                            nc.sync.dma_start(
                                out=yt[
                                    j * 128 : (j + 1) * 128,
                                    c * CHUNK : (c + 1) * CHUNK,
                                ],
                                in_=o_sb[:],
                            )
                        else:
                            # Very last group: evict + store as two halves on
                            # two queues so the final transfers overlap.
                            cuts = [0, 192, 384, CHUNK]
                            engs = [nc.gpsimd, nc.sync, nc.scalar]
                            for (lo, hi2), eng in zip(zip(cuts, cuts[1:]), engs):
                                nc.scalar.activation(
                                    o_sb[:, lo:hi2],
                                    acc[:, lo:hi2],
                                    AF.Identity,
                                    bias=bias_sb[:, j : j + 1],
                                )
                                eng.dma_start(
                                    out=yt[
                                        j * 128 : (j + 1) * 128,
                                        c * CHUNK + lo : c * CHUNK + hi2,
                                    ],
                                    in_=o_sb[:, lo:hi2],
                                )

    nc.compile()
    return nc


_NC_CACHE = None
_last_in_maps = None


def _get_nc():
    global _NC_CACHE
    if _NC_CACHE is None:
        _NC_CACHE = _build_nc()
    return _NC_CACHE


def kernel(x: np.ndarray, gegenbauer_coeffs: np.ndarray, **unused) -> np.ndarray:
    x = np.asarray(x, dtype=np.float32).reshape(B, I)
    coeffs = np.asarray(gegenbauer_coeffs, dtype=np.float32)

    # Host prep: basis change (exact integers, applied in fp64) and layouts.
    M = _basis_matrix()
    v = np.einsum("iod,dk->kio", coeffs.astype(np.float64), M)
    wv_np = np.ascontiguousarray(
        v[1:].reshape(NK * I, O).astype(np.float32).astype(ml_dtypes.bfloat16)
    )
    # Degree-0 term: y += sum_i V[i,o,0]; exact in fp64.
    bias_np = np.ascontiguousarray(
        v[0].sum(axis=0).astype(np.float32).reshape(OT, 128).T
    )
    xt_full = np.ascontiguousarray(x.T.astype(np.float16))  # [I, B], fp16: x rounding adds only ~5e-4 rel err

    in_maps = []
    for c in range(N_CORES):
        xt_c = np.ascontiguousarray(xt_full[:, c * B_LOC : (c + 1) * B_LOC])
        in_maps.append({"xt": xt_c, "wv": wv_np, "bias_in": bias_np})

    global _last_in_maps
    _last_in_maps = in_maps

    nc = _get_nc()
    try:
        res = run_bass_kernel_spmd(nc, in_maps, core_ids=list(range(N_CORES)))
    except Exception:
        # A previous crashed session can leave a core unrecoverable until
        # the runtime resets it; one retry clears it.
        res = run_bass_kernel_spmd(nc, in_maps, core_ids=list(range(N_CORES)))

    y = np.empty((B, O), dtype=np.float32)
    for c in range(N_CORES):
        y[c * B_LOC : (c + 1) * B_LOC, :] = res.results[c]["yt"].T.astype(np.float32)
    return y


# revision 26
# speedup vs baseline: 1.1966x; 1.0268x over previous
"""GegenbauerKAN layer (alpha=1 -> Chebyshev-U basis) on 8 TRN2 NeuronCores.

Math: y[b,o] = sum_{i,d} C_d(tanh(x[b,i])) * W[i,o,d],  d=0..7,
where C_d are Gegenbauer(alpha=1) = Chebyshev-U polynomials.

Strategy:
  - Data-parallel over batch: each of the 8 cores handles 2048 rows.
  - Transposed layout: the host feeds x^T slices so the contraction
    index i lives on SBUF partitions with no on-device transposes.
  - On-device basis: exact U_d values via Chebyshev addition formulas,
    computed in fp32 with bf16 copies at the matmul boundary:
        t  = tanh(x)            s4 = (2t)^2 = U2+1
        b3 = (s4-2)t = U3/2     q2 = (s4-1)^2 = U2^2
        b4 = q2-s4   = U4       b5 = (s4-2)b3 = (U5+2t)/2
        q3 = b3^2               b6 = 4q3-q2 = U6
        e4 = (s4-2)^2           b7 = (e4-2)b3 = U7/2
  - Matmuls in bf16 (1 cycle/row on the PE, same as f32r, but half the
    weight DMA bytes, which is what the pipeline head is bound by, and
    cheaper LDWEIGHTS so weight loads hide fully under the matmuls).
    x ships as fp16 (adds only ~5e-4 rel err; halves the x stream).
    End-to-end rel err ~2.8e-3 vs the 2e-2 gate.
  - k=0 (U_0 = 1) is folded into a per-output bias computed exactly on
    the HOST in fp64 and DMA'd in as a tiny [128, OT] tensor, added at
    PSUM eviction.
  - k-outer matmul order: all four output-tile PSUM groups accumulate
    simultaneously, so weight tile k is first needed ~3.8us * (k-1) in
    and the weight stream can never stall the PE.
  - Head DMA schedule: per-queue transfers FIFO at ~45-100 GB/s while
    the queues share HBM with the other 7 cores' identical fetch
    pattern, so the critical first transfers are spread over the three
    DMA-capable queues: gpsimd[x0a0, wv1(a01), wv2..wv7],
    sync[wv1(a23), x0a1, x0a3, x1..x3, y stores], scalar[bias, x0a2].
    Chunk 0 consumes a-tiles in arrival order [0,2,1,3] and its basis
    DAG is emitted per-a-subtile with the bf16 tensors computed
    directly from their inputs (1-2 op latency, identical values).
  - Later x chunks live in a bufs=1 pool: chunk c+1's x DMA descriptor
    only issues after chunk c's tanh reads, keeping the x stream off
    the HBM while the weights race in at the head.
  - Junk matmuls at t=0 bridge the HAM p-state window (PE at 1.2 GHz
    until ~3.4us of sustained activity) so real matmuls run at 2.4 GHz.
  - Last chunk runs j-outer with immediate per-j eviction so only one
    [128,512] eviction + store sits in the pipeline tail.
  - Weights basis change on host: y = sum_k phi_k . V_k with
    V[:,:,k] = sum_d W[:,:,d] M[d,k], M the (exact, tiny) change of
    basis from {phi_k} to {U_d}; applied in fp64, rounded once.
"""

import ml_dtypes
import numpy as np

import concourse.bacc as bacc
import concourse.mybir as mybir
import concourse.tile as tile
from concourse.alu_op_type import AluOpType as ALU
from concourse.bass_utils import run_bass_kernel_spmd

F32 = mybir.dt.float32
F16 = mybir.dt.float16
BF16 = mybir.dt.bfloat16
AF = mybir.ActivationFunctionType

N_CORES = 8
B = 16384
I = 512
O = 512
DEG = 8  # degrees 0..7
NK = DEG - 1  # degrees 1..7 computed on device; degree 0 is the host bias
B_LOC = B // N_CORES  # 2048 rows per core
CHUNK = 512  # b columns processed per pipeline stage
N_CHUNKS = B_LOC // CHUNK
IT = I // 128  # 4 partition tiles of the input-feature dim
OT = O // 128  # 4 partition tiles of the output dim
N_WARM = 28  # p-state warm-up matmuls (256 cols each) bridging the
# engine-program start to first-operand arrival (~6-10us, run-varying);
# overshoot costs little since chunk 0 is weight-stream-paced anyway
A0_ORDER = [0, 2, 1, 3]  # chunk-0 a-tile consumption = DMA arrival order


def _basis_matrix() -> np.ndarray:
    """M[d,k]: U_d = sum_k M[d,k] * phi_k for the on-device basis
    phi = [1, t, s4, b3, b4, b5, b6, b7]."""
    M = np.zeros((DEG, DEG))
    M[0, 0] = 1.0
    M[1, 1] = 2.0  # U1 = 2 t
    M[2, 0] = -1.0
    M[2, 2] = 1.0  # U2 = s4 - 1
    M[3, 3] = 2.0  # U3 = 2 b3
    M[4, 4] = 1.0  # U4 = b4
    M[5, 5] = 2.0
    M[5, 1] = -2.0  # U5 = 2 b5 - 2 t
    M[6, 6] = 1.0  # U6 = b6
    M[7, 7] = 2.0  # U7 = 2 b7
    return M


def _build_nc():
    nc = bacc.Bacc("TRN2", target_bir_lowering=False, debug=False)

    xt = nc.dram_tensor("xt", [I, B_LOC], F16, kind="ExternalInput")
    wv = nc.dram_tensor("wv", [NK * I, O], BF16, kind="ExternalInput")
    bias_in = nc.dram_tensor("bias_in", [128, OT], F32, kind="ExternalInput")
    yt = nc.dram_tensor("yt", [O, B_LOC], BF16, kind="ExternalOutput")

    with tile.TileContext(nc) as tc:
        with (
            tc.tile_pool(name="wvp", bufs=1) as wvp,
            tc.tile_pool(name="sb", bufs=1) as sb,
            # bufs=1 is deliberate: chunk c+1's x DMA descriptor can only
            # issue after chunk c's tanh reads, which keeps the x stream
            # off the HBM while the weight tiles race in at the head.
            tc.tile_pool(name="xtp", bufs=1) as xtp,
            tc.tile_pool(name="outp", bufs=4) as outp,
            tc.tile_pool(name="ps", bufs=8, space="PSUM") as ps,
        ):
            # --- t=0 setup: constants + warm-up source on the vector queue
            # (memset cannot target bf16/f32r reliably; memset f32, cast)
            wz_f = sb.tile([128, 256], F32, tag="wz_f")
            nc.vector.memset(wz_f[:], 0.0)
            wz = sb.tile([128, 256], BF16, tag="wz")
            nc.vector.tensor_copy(wz[:], wz_f[:])
            neg1 = sb.tile([128, 1], F32, tag="neg1")
            nc.vector.memset(neg1[:], -1.0)
            neg2 = sb.tile([128, 1], F32, tag="neg2")
            nc.vector.memset(neg2[:], -2.0)

            # Weight tiles (bf16). wv1 lands as two halves on two queues;
            # wv2..wv7 stream whole-tile on gpsimd, arriving just ahead
            # of the k-outer consumer (0.5 MiB / ~3.5us each).
            wv_sb = [None] * (NK + 1)
            for k in range(1, NK + 1):
                wv_sb[k] = wvp.tile(
                    [128, IT, O], BF16, tag=f"wv{k}", name=f"wv{k}_sb"
                )

            def wv_dma(eng, k, alo, ahi):
                r0 = (k - 1) * I + alo * 128
                r1 = (k - 1) * I + ahi * 128
                eng.dma_start(
                    out=wv_sb[k][:, alo:ahi, :],
                    in_=wv[r0:r1, :].rearrange("(a p) o -> p a o", p=128),
                )

            # Only sync/scalar/gpsimd queues can issue DMAs, with
            # per-queue FIFO streams. The two most latency-critical
            # transfers — x0's a0 slice and wv1's first half — lead the
            # gpsimd queue; the rest of x0 rides sync/scalar so no
            # single queue serializes the whole head.
            bias_sb = sb.tile([128, OT], F32, tag="bias")
            x0_sb = xtp.tile([128, IT, CHUNK], F16, tag="xt")

            nc.gpsimd.dma_start(out=x0_sb[:, 0, :], in_=xt[0:128, 0:CHUNK])
            wv_dma(nc.sync, 1, 2, 4)
            nc.scalar.dma_start(out=bias_sb[:], in_=bias_in[:, :])
            nc.scalar.dma_start(out=x0_sb[:, 2, :], in_=xt[256:384, 0:CHUNK])
            wv_dma(nc.gpsimd, 1, 0, 2)
            nc.sync.dma_start(out=x0_sb[:, 1, :], in_=xt[128:256, 0:CHUNK])
            nc.sync.dma_start(out=x0_sb[:, 3, :], in_=xt[384:512, 0:CHUNK])
            for k in range(2, NK + 1):
                wv_dma(nc.gpsimd, k, 0, IT)

            # PE p-state warm-up: junk matmuls on zeros, result never read.
            warm_acc = ps.tile([128, 256], F32, tag="acc")
            for i in range(N_WARM):
                nc.tensor.matmul(
                    warm_acc[:],
                    lhsT=wz[:, 0:128],
                    rhs=wz[:],
                    start=(i == 0),
                    stop=(i == N_WARM - 1),
                )

            # Remaining x chunks on sync; the bufs=1 pool gates each one.
            x_tiles = [x0_sb]
            for c in range(1, N_CHUNKS):
                x_sb = xtp.tile([128, IT, CHUNK], F16, tag="xt", name=f"x{c}_sb")
                nc.sync.dma_start(
                    out=x_sb[:],
                    in_=xt[:, c * CHUNK : (c + 1) * CHUNK].rearrange(
                        "(a p) b -> p a b", p=128
                    ),
                )
                x_tiles.append(x_sb)

            flat = [128, IT * CHUNK]

            def dag(x_sb, fine):
                """fp32 basis DAG with bf16 boundary tensors; fine=True
                splits the early ops per a-subtile (in DMA arrival order)
                and computes the bf16 tensors directly from their inputs
                (identical values to the cast-after path, but 1-2 op
                latency) so the PE never waits at the head."""
                t = sb.tile(flat, F32, tag="t")
                t_r = sb.tile(flat, BF16, tag="t_r", bufs=2)
                s4 = sb.tile(flat, F32, tag="s4")
                s4_r = sb.tile(flat, BF16, tag="s4_r", bufs=2)
                b3 = sb.tile(flat, F32, tag="b3")
                b3_r = sb.tile(flat, BF16, tag="b3_r")

                def sl(ap, a):
                    return ap[:] if a is None else ap[:, a * CHUNK : (a + 1) * CHUNK]

                xf = x_sb[:].rearrange("p a b -> p (a b)")

                if fine:
                    A = A0_ORDER
                    # tanh/s4r pairs per a-tile: k=2's operand lands 2 ops
                    # after each x sub-tile arrives.
                    for a in A:
                        nc.scalar.activation(sl(t, a), x_sb[:, a, :], AF.Tanh)
                        nc.scalar.activation(
                            sl(s4_r, a), sl(t, a), AF.Square, scale=2.0
                        )
                    for a in A:
                        nc.scalar.activation(sl(s4, a), sl(t, a), AF.Square, scale=2.0)
                    for a in A:
                        nc.vector.tensor_copy(sl(t_r, a), sl(t, a))
                    for a in A:
                        nc.vector.scalar_tensor_tensor(
                            sl(b3_r, a), sl(s4, a), 2.0, sl(t, a), ALU.subtract, ALU.mult
                        )
                    nc.vector.scalar_tensor_tensor(
                        b3[:], s4[:], 2.0, t[:], ALU.subtract, ALU.mult
                    )
                else:
                    nc.scalar.activation(t[:], xf, AF.Tanh)
                    nc.vector.tensor_copy(t_r[:], t[:])
                    nc.scalar.activation(s4[:], t[:], AF.Square, scale=2.0)
                    nc.vector.tensor_copy(s4_r[:], s4[:])
                    nc.vector.scalar_tensor_tensor(
                        b3[:], s4[:], 2.0, t[:], ALU.subtract, ALU.mult
                    )
                    nc.vector.tensor_copy(b3_r[:], b3[:])
                q2 = sb.tile(flat, F32, tag="q2")
                nc.scalar.activation(q2[:], s4[:], AF.Square, bias=neg1[:])
                b4_r = sb.tile(flat, BF16, tag="b4_r")
                nc.vector.tensor_sub(b4_r[:], q2[:], s4[:])
                b5 = sb.tile(flat, BF16, tag="b5")
                nc.vector.scalar_tensor_tensor(
                    b5[:], s4[:], 2.0, b3[:], ALU.subtract, ALU.mult
                )
                q3 = sb.tile(flat, F32, tag="qe")
                nc.scalar.activation(q3[:], b3[:], AF.Square)
                b6 = sb.tile(flat, BF16, tag="b6")
                nc.vector.scalar_tensor_tensor(
                    b6[:], q3[:], 4.0, q2[:], ALU.mult, ALU.subtract
                )
                e4 = sb.tile(flat, F32, tag="qe")
                nc.scalar.activation(e4[:], s4[:], AF.Square, bias=neg2[:])
                b7 = sb.tile(flat, BF16, tag="b7")
                nc.vector.scalar_tensor_tensor(
                    b7[:], e4[:], 2.0, b3[:], ALU.subtract, ALU.mult
                )
                return [t_r, s4_r, b3_r, b4_r, b5, b6, b7]  # k = 1..7

            pending = []

            def emit_evictions():
                # Evict the previous chunk's PSUM groups. Emitted AFTER the
                # next chunk's basis DAG so the strict-FIFO ACT queue
                # prioritizes producing the basis the PE is waiting on.
                for c0, j, acc in pending:
                    o_sb = outp.tile([128, CHUNK], BF16, tag="out")
                    nc.scalar.activation(
                        o_sb[:], acc[:], AF.Identity, bias=bias_sb[:, j : j + 1]
                    )
                    nc.sync.dma_start(
                        out=yt[j * 128 : (j + 1) * 128, c0 * CHUNK : (c0 + 1) * CHUNK],
                        in_=o_sb[:],
                    )
                pending.clear()

            n_mm = NK * IT
            for c in range(N_CHUNKS):
                basis = dag(x_tiles[c], fine=(c == 0))
                emit_evictions()
                a_order = A0_ORDER if c == 0 else list(range(IT))
                if c < N_CHUNKS - 1:
                    # k-outer: all four output tiles accumulate at once, so
                    # weight tile k is first needed ~3.8us * (k-1) in.
                    accs = [
                        ps.tile([128, CHUNK], F32, tag="acc", name=f"acc_{c}_{j}")
                        for j in range(OT)
                    ]
                    for k in range(1, NK + 1):
                        pk = basis[k - 1]
                        for a in a_order:
                            for j in range(OT):
                                nc.tensor.matmul(
                                    accs[j][:],
                                    lhsT=wv_sb[k][:, a, j * 128 : (j + 1) * 128],
                                    rhs=pk[:, a * CHUNK : (a + 1) * CHUNK],
                                    start=(k == 1 and a == a_order[0]),
                                    stop=(k == NK and a == a_order[-1]),
                                )
                    pending.extend((c, j, accs[j]) for j in range(OT))
                else:
                    # Last chunk: j-outer with immediate eviction so the
                    # tail after the final matmul is one evict + one store.
                    for j in range(OT):
                        acc = ps.tile([128, CHUNK], F32, tag="acc", name=f"acc_{c}_{j}")
                        idx = 0
                        for k in range(1, NK + 1):
                            pk = basis[k - 1]
                            for a in a_order:
                                nc.tensor.matmul(
                                    acc[:],
                                    lhsT=wv_sb[k][:, a, j * 128 : (j + 1) * 128],
                                    rhs=pk[:, a * CHUNK : (a + 1) * CHUNK],
                                    start=(idx == 0),
                                    stop=(idx == n_mm - 1),
                                )
                                idx += 1
                        o_sb = outp.tile([128, CHUNK], BF16, tag="out")
                        if j < OT - 1:
                            nc.scalar.activation(
                                o_sb[:], acc[:], AF.Identity, bias=bias_sb[:, j : j + 1]
                            )
                            nc.sync.dma_start(
                                out=yt[
                                    j * 128 : (j + 1) * 128,
                                    c * CHUNK : (c + 1) * CHUNK,
                                ],
                                in_=o_sb[:],
                            )
                        else:
                            # Very last group: evict + store as two halves on
                            # two queues so the final transfers overlap.
                            h = CHUNK // 2
                            for hi, eng in ((0, nc.sync), (1, nc.scalar)):
                                sl_ = slice(hi * h, (hi + 1) * h)
                                nc.scalar.activation(
                                    o_sb[:, sl_],
                                    acc[:, sl_],
                                    AF.Identity,
                                    bias=bias_sb[:, j : j + 1],
                                )
                                eng.dma_start(
                                    out=yt[
                                        j * 128 : (j + 1) * 128,
                                        c * CHUNK + hi * h : c * CHUNK + (hi + 1) * h,
                                    ],
                                    in_=o_sb[:, sl_],
                                )

    nc.compile()
    return nc


_NC_CACHE = None
_last_in_maps = None


def _get_nc():
    global _NC_CACHE
    if _NC_CACHE is None:
        _NC_CACHE = _build_nc()
    return _NC_CACHE


def kernel(x: np.ndarray, gegenbauer_coeffs: np.ndarray, **unused) -> np.ndarray:
    x = np.asarray(x, dtype=np.float32).reshape(B, I)
    coeffs = np.asarray(gegenbauer_coeffs, dtype=np.float32)

    # Host prep: basis change (exact integers, applied in fp64) and layouts.
    M = _basis_matrix()
    v = np.einsum("iod,dk->kio", coeffs.astype(np.float64), M)
    wv_np = np.ascontiguousarray(
        v[1:].reshape(NK * I, O).astype(np.float32).astype(ml_dtypes.bfloat16)
    )
    # Degree-0 term: y += sum_i V[i,o,0]; exact in fp64.
    bias_np = np.ascontiguousarray(
        v[0].sum(axis=0).astype(np.float32).reshape(OT, 128).T
    )
    xt_full = np.ascontiguousarray(x.T.astype(np.float16))  # [I, B], fp16: x rounding adds only ~5e-4 rel err

    in_maps = []
    for c in range(N_CORES):
        xt_c = np.ascontiguousarray(xt_full[:, c * B_LOC : (c + 1) * B_LOC])
        in_maps.append({"xt": xt_c, "wv": wv_np, "bias_in": bias_np})

    global _last_in_maps
    _last_in_maps = in_maps

    nc = _get_nc()
    try:
        res = run_bass_kernel_spmd(nc, in_maps, core_ids=list(range(N_CORES)))
    except Exception:
        # A previous crashed session can leave a core unrecoverable until
        # the runtime resets it; one retry clears it.
        res = run_bass_kernel_spmd(nc, in_maps, core_ids=list(range(N_CORES)))

    y = np.empty((B, O), dtype=np.float32)
    for c in range(N_CORES):
        y[c * B_LOC : (c + 1) * B_LOC, :] = res.results[c]["yt"].T.astype(np.float32)
    return y
